# revision 51
# baseline (speedup 1.0000x reference)
"""Sliding-window causal self-attention (GQA + RoPE + QK-RMSNorm + ve-gate) on
8 Trainium2 NeuronCores.

Sharding: core c handles (batch b = c // 4, kv-head g = c % 4): data parallel
over batch x tensor parallel over the 4 KV head groups (4 query heads per
core). Each core computes its partial c_proj output; the all-reduce over the 4
head shards is a host-side sum.

Device design (per core), v3.0:
  - x, Wq/Wk/Wv/Wg/Wproj are fed in bf16 (halves DMA + SBUF; matmul rate is
    unchanged and PSUM accumulation stays fp32). kT/q4 (the score inner
    product) stay float32r; the post-softmax path (es/vn/masks/ones) is bf16,
    which also enables the DVE 2x mode for the mask multiplies.
  - q for all 4 heads lives in ONE SBUF tile q4 [128, 4, TS]; scores / ones /
    PV matmuls process all 4 heads per k-tile with [128, 4*128] outputs, so
    every fp32r matmul has a moving free-size of 512 (full PE rate) and the
    instruction count is 1/4 of the per-head variant.
  - scores are computed TRANSPOSED (S^T: tk x tq) so P@V needs no transposes.
  - softmax denominators are HYBRID: the first L-3 k-tiles of each q-subtile
    are partition-summed on GpSimd (tensor_reduce axis=C over groups of 3,
    partials parked at partition 32*g of a bf16 `rows` tile and folded in by
    sel-matmuls); only the last 3 k-tiles use PE ones-matmuls so the
    denominator dependency chain stays short.
  - softmax skips max-subtraction: QK RMS-norm bounds |scores| so exp() cannot
    overflow in fp32. Sliding-window masking multiplies the two triangular
    boundary tiles by 0/1 masks (btri4/etri4, pre-replicated over heads) on
    the Vector engine, ordered early-middle in the tile list so their longer
    chains overlap the remaining full tiles.
  - RoPE's half-swap runs as a PE permutation matmul (no DMA latency); both
    q's and k's rms-norm scales ride the final rope multiply, with
    rsqrt(mean-square) computed via gpsimd.partition_all_reduce (output
    already broadcast across partitions) + Ln/Exp activations.
  - c_proj runs at q-subtile granularity (moving operand yt4 is bf16, so
    128-column matmuls still run at 1 cycle/row) and is emitted interleaved
    into the NEXT attention subtile as PE filler work; each co-group's PSUM
    is evacuated (ACT/DVE alternating) and DMA'd out immediately.
  - the NEXT slice's QKV phases (k at j=0, q-head pairs at j=1/2, v at j=3)
    are emitted INSIDE the attention subtile loop: the PE runs in order, so
    without this the ACT-paced attention stretch would leave the PE starved
    once the c_proj filler is consumed.
"""

import sys

sys.path.insert(0, "/opt/trn_rl_repo")

import numpy as np

B, T, C = 2, 2048, 2048
NH, NKV, HD = 16, 4, 128
GATE_CH = 12
HPC = NH // NKV          # q heads per core
TS = 512                 # token-slice width
NSL = T // TS            # 4 slices
NCK = C // 128           # 16 contraction chunks
TPS = TS // 128          # 4 token tiles per slice
NTT = T // 128           # 16 token tiles
EPS = 1e-6

A_Q = 1.2 / np.sqrt(float(HD))   # rms-norm scale folded into q (incl 1/sqrt(HD))
A_K = 1.2                        # rms-norm scale folded into exp() scale arg
S_Q = float(1.0 / (HD * A_Q * A_Q))
B_Q = float(EPS / (A_Q * A_Q))
S_K = float(1.0 / (HD * A_K * A_K))
B_K = float(EPS / (A_K * A_K))
LN3I = float(np.log(1.0 / 3.0))

_compiled = {}


def _build(W):
    import concourse.bass as bass
    import concourse.tile as tile
    from concourse import bacc, bass_isa, mybir
    from concourse.masks import make_identity
    from contextlib import ExitStack

    f32 = mybir.dt.float32
    f32r = mybir.dt.float32r
    bf16 = mybir.dt.bfloat16
    AF = mybir.ActivationFunctionType
    OP = mybir.AluOpType

    NW = W // 128            # window in 128-tiles (8)
    assert W % 128 == 0

    nc = bacc.Bacc(None, target_bir_lowering=False)

    xT = nc.dram_tensor("xT", [C, T], bf16, kind="ExternalInput")
    wq = nc.dram_tensor("wqT", [C, HPC * HD], bf16, kind="ExternalInput")
    wk = nc.dram_tensor("wkT", [C, HD], bf16, kind="ExternalInput")
    wv = nc.dram_tensor("wvT", [C, HD], bf16, kind="ExternalInput")
    wp = nc.dram_tensor("wpT", [HPC * HD, C], bf16, kind="ExternalInput")
    wgd = nc.dram_tensor("wg", [GATE_CH, 1], bf16, kind="ExternalInput")
    ccd = nc.dram_tensor("cc", [HD, T], f32, kind="ExternalInput")
    ssd = nc.dram_tensor("ss", [HD, T], f32, kind="ExternalInput")
    ved = nc.dram_tensor("ve", [T, HD], f32, kind="ExternalInput")
    btrid = nc.dram_tensor("btri4", [128, HPC * 128], bf16, kind="ExternalInput")
    etrid = nc.dram_tensor("etri4", [128, HPC * 128], bf16, kind="ExternalInput")
    permd = nc.dram_tensor("perm", [128, 128], f32r, kind="ExternalInput")
    seld = nc.dram_tensor("sel", [128, 3], bf16, kind="ExternalInput")
    outT = nc.dram_tensor("outT", [C, T], f32, kind="ExternalOutput")

    with tile.TileContext(nc) as tc, ExitStack() as ctx:
        res = ctx.enter_context(tc.tile_pool(name="res", bufs=1))
        xs_p = ctx.enter_context(tc.tile_pool(name="xs", bufs=2))
        tab_p = ctx.enter_context(tc.tile_pool(name="tab", bufs=2))
        work_p = ctx.enter_context(tc.tile_pool(name="work", bufs=2))
        sq_p = ctx.enter_context(tc.tile_pool(name="sq", bufs=2))
        bc_p = ctx.enter_context(tc.tile_pool(name="bc", bufs=2))
        es_p = ctx.enter_context(tc.tile_pool(name="es", bufs=3))
        es3_p = ctx.enter_context(tc.tile_pool(name="es3", bufs=2))
        q4_p = ctx.enter_context(tc.tile_pool(name="q4", bufs=2))
        yt_p = ctx.enter_context(tc.tile_pool(name="yt", bufs=2))
        ot_p = ctx.enter_context(tc.tile_pool(name="ot", bufs=2))
        row_p = ctx.enter_context(tc.tile_pool(name="rows", bufs=2))

        ps_qkv = ctx.enter_context(tc.tile_pool(name="ps_qkv", bufs=2, space="PSUM"))
        ps_s = ctx.enter_context(tc.tile_pool(name="ps_s", bufs=2, space="PSUM"))
        ps_y = ctx.enter_context(tc.tile_pool(name="ps_y", bufs=1, space="PSUM"))
        ps_p = ctx.enter_context(tc.tile_pool(name="ps_p", bufs=2, space="PSUM"))
        ps_row = ctx.enter_context(tc.tile_pool(name="ps_row", bufs=1, space="PSUM"))
        dram_p = ctx.enter_context(tc.tile_pool(name="dram", bufs=2, space="DRAM"))

        # ---- resident tensors ----
        wg_sb = res.tile([GATE_CH, 1], bf16)
        nc.sync.dma_start(out=wg_sb, in_=wgd[:, :])
        wk_sb = res.tile([128, NCK, HD], bf16)
        xs0 = xs_p.tile([128, NCK, TS], bf16, tag="xs")
        wq_sb = res.tile([128, NCK, HPC * HD], bf16)
        # first parts small so the first k-proj matmuls can start early;
        # wq parts interleave so the q projections can start right after k
        def ldx(c0, c1):
            nc.sync.dma_start(
                out=xs0[:, c0:c1, :],
                in_=xT[128 * c0:128 * c1, 0:TS].rearrange(
                    "(c p) t -> p c t", p=128),
            )
        def ldwq(c0, c1):
            nc.sync.dma_start(
                out=wq_sb[:, c0:c1, :],
                in_=wq[128 * c0:128 * c1, :].rearrange(
                    "(c p) h -> p c h", p=128),
            )
        def ldwk(c0, c1):
            nc.sync.dma_start(
                out=wk_sb[:, c0:c1, :],
                in_=wk[128 * c0:128 * c1, :].rearrange(
                    "(c p) h -> p c h", p=128))
        ldwk(0, 1)
        ldx(0, 1)
        ldwk(1, 6)
        ldx(1, 6)
        ldwk(6, 16)
        ldx(6, 11)
        ldwq(0, 3)
        ldx(11, 16)
        ldwq(3, 8)
        ldwq(8, 12)
        ldwq(12, 16)
        wv_sb = res.tile([128, NCK, HD], bf16)
        nc.sync.dma_start(
            out=wv_sb, in_=wv[:, :].rearrange("(c p) h -> p c h", p=128))
        cc0 = tab_p.tile([128, TS], f32, tag="cc")
        nc.sync.dma_start(out=cc0, in_=ccd[:, 0:TS])
        ss0 = tab_p.tile([128, TS], f32, tag="ss")
        nc.sync.dma_start(out=ss0, in_=ssd[:, 0:TS])
        ve0 = tab_p.tile([128, TPS, HD], f32, tag="ve")
        nc.sync.dma_start(
            out=ve0, in_=ved[0:TS, :].rearrange("(tt p) h -> p tt h", p=128))
        btri_sb = res.tile([128, HPC * 128], bf16)
        nc.sync.dma_start(out=btri_sb, in_=btrid[:, :])
        etri_sb = res.tile([128, HPC * 128], bf16)
        nc.sync.dma_start(out=etri_sb, in_=etrid[:, :])
        perm_sb = res.tile([128, 128], f32r)
        nc.sync.dma_start(out=perm_sb, in_=permd[:, :])
        sel_sb = res.tile([128, 3], bf16)
        nc.sync.dma_start(out=sel_sb, in_=seld[:, :])
        wp_sb = res.tile([128, HPC, C], bf16)
        for h in range(HPC):
            nc.sync.dma_start(out=wp_sb[:, h, :], in_=wp[h * 128:(h + 1) * 128, :])

        ident = res.tile([128, 128], f32)
        make_identity(nc, ident)
        ones_sb = res.tile([128, 1], bf16)
        nc.vector.memset(ones_sb, 1.0)
        bq_sb = res.tile([128, 1], f32)
        nc.vector.memset(bq_sb, B_Q)
        bk_sb = res.tile([128, 1], f32)
        nc.vector.memset(bk_sb, B_K)
        bg_sb = res.tile([1, 1], f32)
        nc.vector.memset(bg_sb, LN3I)

        rows_ab = []
        for i in range(2):
            r = res.tile([128, 3, TS], bf16, tag=f"rows{i}")
            nc.vector.memset(r, 0.0)
            rows_ab.append(r)
        kT_sb = res.tile([128, T], f32r)        # rotated+normalized k, HD on partitions
        vn_sb = res.tile([128, NTT, HD], bf16)  # v natural, token tiles on partitions

        tabs = {0: (cc0, ss0, ve0)}
        xss = {0: xs0}
        yt_tiles = {}
        q4s = {}
        gates = {}

        def rope_half(dst_f32r, cc_sl, ss_sl, tag, scale_bc=None):
            """dst [128, TS] f32r pre-rotation. In-place RoPE; the half-swap
            runs as a PE permutation matmul (no DMA latency). The final write
            goes through the f32r view (required by consuming f32r matmuls)."""
            dst = dst_f32r.bitcast(f32)
            psw = ps_s.tile([128, HPC * 128], f32, tag="s")
            nc.tensor.matmul(psw[:, 0:TS], perm_sb, dst_f32r,
                             start=True, stop=True)
            tmp = work_p.tile([128, TS], f32, tag=tag + "t")
            nc.vector.tensor_mul(tmp, psw[:, 0:TS], ss_sl)
            nc.vector.tensor_mul(dst_f32r, dst, cc_sl)
            nc.vector.tensor_add(dst_f32r, dst, tmp)
            if scale_bc is not None:
                nc.vector.tensor_mul(dst_f32r, dst, scale_bc)

        def emit_cproj(m, j):
            """c_proj for q-subtile j of slice m (yt_tiles[(m, j)] ready)."""
            yt4 = yt_tiles.pop((m, j))
            t0 = m * TS
            ot = ot_p.tile([128, NTT, 128], f32, tag="ot")
            for gco in range(4):
                pp = ps_p.tile([128, 4 * 128], f32, tag="pp")
                for ci in range(4):
                    co = 4 * gco + ci
                    for h in range(HPC):
                        nc.tensor.matmul(
                            pp[:, ci * 128:(ci + 1) * 128],
                            wp_sb[:, h, co * 128:(co + 1) * 128],
                            yt4[:, h, :],
                            start=(h == 0), stop=(h == HPC - 1))
                if gco % 2 == 0:
                    nc.scalar.activation(ot[:, 4 * gco:4 * gco + 4, :], pp, AF.Copy)
                else:
                    nc.vector.tensor_copy(ot[:, 4 * gco:4 * gco + 4, :], pp)
                nc.sync.dma_start(
                    out=outT[512 * gco:512 * (gco + 1),
                             t0 + j * 128:t0 + (j + 1) * 128].rearrange(
                        "(co p) t -> p co t", p=128),
                    in_=ot[:, 4 * gco:4 * gco + 4, :])

        def emit_k(mm):
            """k projection + rms + rope for slice mm."""
            t0 = mm * TS
            xs = xss[mm]
            cc_sl, ss_sl, _ = tabs[mm]
            ps_k = ps_qkv.tile([128, TS], f32, tag="qkv")
            for c in range(NCK):
                nc.tensor.matmul(ps_k, wk_sb[:, c, :], xs[:, c, :],
                                 start=(c == 0), stop=(c == NCK - 1))
            sq_k = sq_p.tile([128, TS], f32, tag="sq")
            nc.scalar.activation(sq_k, ps_k, AF.Square)
            # rsqrt(mean(k^2)) broadcast across partitions; k is pre-normalized
            # (scale folded into the rope's final multiply), so the exp() scale
            # becomes the constant 1.0.
            rbk = bc_p.tile([128, TS], f32, tag="bc")
            nc.gpsimd.partition_all_reduce(rbk, sq_k, channels=128,
                                           reduce_op=bass_isa.ReduceOp.add)
            nc.scalar.activation(rbk, rbk, AF.Ln, bias=bk_sb, scale=S_K)
            nc.scalar.activation(rbk, rbk, AF.Exp, scale=-0.5)
            k_sl = kT_sb[:, t0:t0 + TS]
            nc.vector.tensor_copy(k_sl, ps_k)
            rope_half(k_sl, cc_sl, ss_sl, "ksw", scale_bc=rbk)

        def emit_gate(mm):
            """gate row: 3*sigmoid(x[:, :12] @ wg)."""
            xs = xss[mm]
            ps_g = ps_row.tile([1, TS], f32, tag="rows")
            nc.tensor.matmul(ps_g, wg_sb, xs[0:GATE_CH, 0, :], start=True,
                             stop=True)
            g_row = row_p.tile([1, TS], f32, tag="grow")
            # e^(-x)/3, then +1/3, then reciprocal => 3*sigmoid(x)
            nc.scalar.activation(g_row, ps_g, AF.Exp, scale=-1.0, bias=bg_sb)
            nc.vector.tensor_scalar(out=g_row, in0=g_row, scalar1=1.0 / 3.0,
                                    scalar2=None, op0=OP.add)
            nc.vector.reciprocal(g_row, g_row)
            g_dr = dram_p.tile([TS], f32, tag="gdr")
            nc.sync.dma_start(out=g_dr, in_=g_row)
            gate_c = row_p.tile([128, TPS], f32, tag="gate")
            nc.sync.dma_start(
                out=gate_c,
                in_=bass.AP(tensor=g_dr.tensor, offset=g_dr.offset,
                            ap=[[1, 128], [128, TPS]]),
            )
            gates[mm] = gate_c

        def emit_q(mm, h):
            """one q head: projection + rms-norm + rope. The rms-norm scale is
            applied as the LAST rope step so the rotation can proceed in
            parallel with the row chain."""
            xs = xss[mm]
            cc_sl, ss_sl, _ = tabs[mm]
            if h == 0:
                q4new = q4_p.tile([128, HPC, TS], f32r, tag="q4")
                q4s[mm] = q4new
            q4 = q4s[mm]
            ps_q = ps_qkv.tile([128, TS], f32, tag="qkv")
            for c in range(NCK):
                nc.tensor.matmul(ps_q, wq_sb[:, c, h * HD:(h + 1) * HD],
                                 xs[:, c, :],
                                 start=(c == 0), stop=(c == NCK - 1))
            nc.vector.tensor_copy(q4[:, h, :], ps_q)
            sq_q = sq_p.tile([128, TS], f32, tag="sq")
            nc.scalar.activation(sq_q, ps_q, AF.Square)
            rbc = bc_p.tile([128, TS], f32, tag="bc")
            nc.gpsimd.partition_all_reduce(rbc, sq_q,
                                           channels=128,
                                           reduce_op=bass_isa.ReduceOp.add)
            nc.scalar.activation(rbc, rbc, AF.Ln, bias=bq_sb, scale=S_Q)
            nc.scalar.activation(rbc, rbc, AF.Exp, scale=-0.5)
            rope_half(q4[:, h, :], cc_sl, ss_sl, "qsw", scale_bc=rbc)

        def emit_v(mm):
            """v projection + transpose to natural + gate-add; last user of
            xs/tabs/gate for slice mm."""
            xs = xss.pop(mm)
            _, _, ve_sl = tabs.pop(mm)
            gate_c = gates.pop(mm)
            ps_v = ps_qkv.tile([128, TS], f32, tag="qkv")
            for c in range(NCK):
                nc.tensor.matmul(ps_v, wv_sb[:, c, :], xs[:, c, :],
                                 start=(c == 0), stop=(c == NCK - 1))
            vT_s = work_p.tile([128, TS], f32, tag="vt")
            nc.scalar.activation(vT_s, ps_v, AF.Copy)
            ps_t = ps_qkv.tile([128, TS], f32, tag="qkv")
            for tt in range(TPS):
                nc.tensor.transpose(ps_t[:, tt * 128:(tt + 1) * 128],
                                    vT_s[:, tt * 128:(tt + 1) * 128], ident)
            for tt in range(TPS):
                nc.vector.scalar_tensor_tensor(
                    out=vn_sb[:, mm * TPS + tt, :],
                    in0=ve_sl[:, tt, :], scalar=gate_c[:, tt:tt + 1],
                    in1=ps_t[:, tt * 128:(tt + 1) * 128],
                    op0=OP.mult, op1=OP.add)

        def emit_prefetch(mm):
            """issue the xs/cc/ss/ve loads for slice mm."""
            t1 = mm * TS
            xs_n = xs_p.tile([128, NCK, TS], bf16, tag="xs")
            for p4 in range(4):
                nc.sync.dma_start(
                    out=xs_n[:, 4 * p4:4 * p4 + 4, :],
                    in_=xT[512 * p4:512 * (p4 + 1), t1:t1 + TS].rearrange(
                        "(c p) t -> p c t", p=128),
                )
            cc_n = tab_p.tile([128, TS], f32, tag="cc")
            nc.sync.dma_start(out=cc_n, in_=ccd[:, t1:t1 + TS])
            ss_n = tab_p.tile([128, TS], f32, tag="ss")
            nc.sync.dma_start(out=ss_n, in_=ssd[:, t1:t1 + TS])
            ve_n = tab_p.tile([128, TPS, HD], f32, tag="ve")
            nc.sync.dma_start(
                out=ve_n,
                in_=ved[t1:t1 + TS, :].rearrange("(tt p) h -> p tt h", p=128))
            xss[mm] = xs_n
            tabs[mm] = (cc_n, ss_n, ve_n)

        # ---- slice 0 QKV up front (nothing to interleave into yet) ----
        emit_k(0)
        emit_gate(0)
        for h in range(HPC):
            emit_q(0, h)
        emit_prefetch(1)
        emit_v(0)

        for m in range(NSL):
            t0 = m * TS
            q4 = q4s[m]

            # ---- attention subtiles, interleaved with c_proj of the previous
            # subtile and the NEXT slice's QKV phases (PE filler during the
            # ACT-paced attention stretch) ----
            for j in range(TPS):
                t = m * TPS + j
                nlo = max(0, t - NW)
                # Order: one full tile opens the PSUM groups (short dep chain),
                # the masked boundary tiles (diag/edge) go next so their longer
                # exp->mask chains overlap the remaining full tiles' work.
                fulls = [n for n in range(nlo, t + 1)
                         if n != t and n != t - NW]
                ns = []
                if fulls:
                    ns.append(fulls[0])
                ns.append(t)                      # diag (btri)
                if t - NW >= 0:
                    ns.append(t - NW)             # edge (etri)
                ns.extend(fulls[1:])
                L = len(ns)
                last = L - 1
                # hybrid denominator: the first `early` tiles are summed on
                # Pool (their reduces finish well before the subtile ends);
                # the last 3 tiles keep PE ones-matmuls so the chain stays
                # short. sel-matmuls fold the Pool partials into ps_sum.
                early = L - 3 if L >= 6 else 0
                G = (early + 2) // 3
                gsz = [min(3, early - 3 * g) for g in range(G)]
                rows = rows_ab[(m * TPS + j) % 2]
                psy = ps_y.tile([128, HPC * 128], f32, tag="py")
                ps_sum = ps_row.tile([1, TS], f32, tag="rows")
                esg = []
                for _g in range(G):
                    es3 = es3_p.tile([128, 3, HPC * 128], bf16, tag="es3")
                    esg.append(es3)
                for idx, n in enumerate(ns):
                    pss = ps_s.tile([128, HPC * 128], f32, tag="s")
                    nc.tensor.matmul(pss, kT_sb[:, n * 128:(n + 1) * 128],
                                     q4[:, :, j * 128:(j + 1) * 128],
                                     start=True, stop=True)
                    if idx < early:
                        g, gi = idx // 3, idx % 3
                        es = esg[g][:, gi, :]
                    else:
                        es = es_p.tile([128, HPC * 128], bf16, tag="es")
                    nc.scalar.activation(es, pss, AF.Exp)
                    if n == t:
                        nc.vector.tensor_mul(es, es, btri_sb)
                    if n == t - NW:
                        nc.vector.tensor_mul(es, es, etri_sb)
                    if idx >= early:
                        nc.tensor.matmul(ps_sum, ones_sb, es,
                                         start=(idx == early),
                                         stop=(G == 0 and idx == last))
                    nc.tensor.matmul(psy, vn_sb[:, n, :], es,
                                     start=(idx == 0), stop=(idx == last))
                    if idx < early and (idx % 3 == gsz[idx // 3] - 1):
                        g = idx // 3
                        with nc.allow_low_precision(
                                reason="f32r rows: softmax denominator "
                                       "tolerates TF32-width rounding"):
                            nc.gpsimd.tensor_reduce(
                                rows[32 * g:32 * g + 1, 0:gsz[g], :],
                                esg[g][:, 0:gsz[g], :],
                                axis=mybir.AxisListType.C, op=OP.add)
                if G:
                    for b in range(gsz[0]):
                        cnt = sum(1 for x in gsz if x > b)
                        nc.tensor.matmul(ps_sum, sel_sb[:, cnt - 1:cnt],
                                         rows[:, b, :], start=False,
                                         stop=(b == gsz[0] - 1))
                rsum = row_p.tile([1, TS], f32, tag="rsum")
                nc.vector.reciprocal(rsum, ps_sum)
                sbc = bc_p.tile([128, TS], f32, tag="sbc")
                nc.gpsimd.partition_broadcast(sbc, rsum)
                yt4 = yt_p.tile([128, HPC, 128], bf16, tag="yt")
                nc.vector.tensor_mul(
                    yt4.rearrange("p h t -> p (h t)"), psy, sbc)
                yt_tiles[(m, j)] = yt4
                if j > 0:
                    emit_cproj(m, j - 1)
                else:
                    if m > 0:
                        emit_cproj(m - 1, TPS - 1)
                if m + 1 < NSL:
                    if j == 0:
                        emit_k(m + 1)
                        emit_gate(m + 1)
                    elif j == 1:
                        emit_q(m + 1, 0)
                        emit_q(m + 1, 1)
                        if m + 2 < NSL:
                            emit_prefetch(m + 2)
                    elif j == 2:
                        emit_q(m + 1, 2)
                        emit_q(m + 1, 3)
                    else:
                        emit_v(m + 1)
            del q4s[m]

        emit_cproj(NSL - 1, TPS - 1)

    # Restrict the activation-table picker to the one set containing every
    # ACT function we use (exp, ln, square, copy): without this the greedy
    # picker alternates tables, inserting a ~1.3us table load per switch.
    import concourse.hw_specs as hw_specs
    import concourse.bacc as bacc_mod

    orig = hw_specs.get_activation_tables

    def only_combined(arch):
        t = orig(arch)
        return {k: (v if k == "natural_log_exp_and_others" else set())
                for k, v in t.items()}

    hw_specs.get_activation_tables = only_combined
    bacc_mod.get_activation_tables = only_combined
    try:
        nc.compile()
    finally:
        hw_specs.get_activation_tables = orig
        bacc_mod.get_activation_tables = orig
    return nc


def _prep_inputs(x, ve, cos, sin, Wq, Wk, Wv, Wproj, Wgate, W):
    import ml_dtypes

    bf = ml_dtypes.bfloat16
    cosT = np.ascontiguousarray(cos[0, :, 0, :].T)  # (64, T)
    sinT = np.ascontiguousarray(sin[0, :, 0, :].T)
    cc = np.concatenate([cosT, cosT], axis=0).astype(np.float32)
    ss = np.concatenate([sinT, -sinT], axis=0).astype(np.float32)
    p = np.arange(128)[:, None]
    f = np.arange(128)[None, :]
    btri = (p <= f).astype(np.float32)
    etri = (f <= p + (W % 128)).astype(np.float32)
    btri4 = np.ascontiguousarray(np.tile(btri, (1, HPC)))
    etri4 = np.ascontiguousarray(np.tile(etri, (1, HPC)))
    # half-swap permutation: out[p] = in[(p+64) % 128]
    perm = np.zeros((128, 128), dtype=np.float32)
    perm[(np.arange(128) + 64) % 128, np.arange(128)] = 1.0
    sel = np.zeros((128, 3), dtype=np.float32)
    for c in range(3):
        sel[[32 * g for g in range(c + 1)], c] = 1.0

    in_maps = []
    for core in range(8):
        b, g = core // NKV, core % NKV
        hs = slice(g * HPC * HD, (g + 1) * HPC * HD)
        ks = slice(g * HD, (g + 1) * HD)
        in_maps.append({
            "xT": np.ascontiguousarray(x[b].T).astype(bf),
            "wqT": np.ascontiguousarray(Wq[hs, :].T).astype(bf),
            "wkT": np.ascontiguousarray(Wk[ks, :].T).astype(bf),
            "wvT": np.ascontiguousarray(Wv[ks, :].T).astype(bf),
            "wpT": np.ascontiguousarray(Wproj[:, hs].T).astype(bf),
            "wg": np.ascontiguousarray(Wgate[g][:, None]).astype(bf),
            "cc": cc,
            "ss": ss,
            "ve": np.ascontiguousarray(ve[b][:, ks]),
            "btri4": btri4.astype(bf),
            "etri4": etri4.astype(bf),
            "perm": perm,
            "sel": sel.astype(bf),
        })
    return in_maps


def _run(inputs, trace=False):
    from concourse.bass_utils import run_bass_kernel_spmd

    x = np.asarray(inputs["x"], dtype=np.float32)
    ve = np.asarray(inputs["ve"], dtype=np.float32)
    cos = np.asarray(inputs["cos"], dtype=np.float32)
    sin = np.asarray(inputs["sin"], dtype=np.float32)
    Wq = np.asarray(inputs["Wq"], dtype=np.float32)
    Wk = np.asarray(inputs["Wk"], dtype=np.float32)
    Wv = np.asarray(inputs["Wv"], dtype=np.float32)
    Wproj = np.asarray(inputs["Wproj"], dtype=np.float32)
    Wgate = np.asarray(inputs["Wgate"], dtype=np.float32)
    W = int(inputs["window_size"])

    if W not in _compiled:
        _compiled[W] = _build(W)
    nc = _compiled[W]

    in_maps = _prep_inputs(x, ve, cos, sin, Wq, Wk, Wv, Wproj, Wgate, W)
    res = run_bass_kernel_spmd(nc, in_maps, core_ids=list(range(8)), trace=trace)

    out = np.zeros((B, T, C), dtype=np.float32)
    for core in range(8):
        b = core // NKV
        out[b] += res.results[core]["outT"].T
    return out, res


def kernel(**inputs):
    out, _ = _run(inputs, trace=False)
    return out


# revision 52
# speedup vs baseline: 1.0059x; 1.0059x over previous
"""Sliding-window causal self-attention (GQA + RoPE + QK-RMSNorm + ve-gate) on
8 Trainium2 NeuronCores.

Sharding: core c handles (batch b = c // 4, kv-head g = c % 4): data parallel
over batch x tensor parallel over the 4 KV head groups (4 query heads per
core). Each core computes its partial c_proj output; the all-reduce over the 4
head shards is a host-side sum.

Device design (per core), v3.0:
  - x, Wq/Wk/Wv/Wg/Wproj are fed in bf16 (halves DMA + SBUF; matmul rate is
    unchanged and PSUM accumulation stays fp32). kT/q4 (the score inner
    product) stay float32r; the post-softmax path (es/vn/masks/ones) is bf16,
    which also enables the DVE 2x mode for the mask multiplies.
  - q for all 4 heads lives in ONE SBUF tile q4 [128, 4, TS]; scores / ones /
    PV matmuls process all 4 heads per k-tile with [128, 4*128] outputs, so
    every fp32r matmul has a moving free-size of 512 (full PE rate) and the
    instruction count is 1/4 of the per-head variant.
  - scores are computed TRANSPOSED (S^T: tk x tq) so P@V needs no transposes.
  - softmax denominators are HYBRID: the first L-3 k-tiles of each q-subtile
    are partition-summed on GpSimd (tensor_reduce axis=C over groups of 3,
    partials parked at partition 32*g of a bf16 `rows` tile and folded in by
    sel-matmuls); only the last 3 k-tiles use PE ones-matmuls so the
    denominator dependency chain stays short.
  - softmax skips max-subtraction: QK RMS-norm bounds |scores| so exp() cannot
    overflow in fp32. Sliding-window masking multiplies the two triangular
    boundary tiles by 0/1 masks (btri4/etri4, pre-replicated over heads) on
    the Vector engine, ordered early-middle in the tile list so their longer
    chains overlap the remaining full tiles.
  - RoPE's half-swap runs as a PE permutation matmul (no DMA latency); both
    q's and k's rms-norm scales ride the final rope multiply, with
    rsqrt(mean-square) computed via gpsimd.partition_all_reduce (output
    already broadcast across partitions) + Ln/Exp activations.
  - c_proj runs at q-subtile granularity (moving operand yt4 is bf16, so
    128-column matmuls still run at 1 cycle/row) and is emitted interleaved
    into the NEXT attention subtile as PE filler work; each co-group's PSUM
    is evacuated (ACT/DVE alternating) and DMA'd out immediately.
  - the NEXT slice's QKV phases (k at j=0, q-head pairs at j=1/2, v at j=3)
    are emitted INSIDE the attention subtile loop: the PE runs in order, so
    without this the ACT-paced attention stretch would leave the PE starved
    once the c_proj filler is consumed.
"""

import sys

sys.path.insert(0, "/opt/trn_rl_repo")

import numpy as np

B, T, C = 2, 2048, 2048
NH, NKV, HD = 16, 4, 128
GATE_CH = 12
HPC = NH // NKV          # q heads per core
TS = 512                 # token-slice width
NSL = T // TS            # 4 slices
NCK = C // 128           # 16 contraction chunks
TPS = TS // 128          # 4 token tiles per slice
NTT = T // 128           # 16 token tiles
EPS = 1e-6

A_Q = 1.2 / np.sqrt(float(HD))   # rms-norm scale folded into q (incl 1/sqrt(HD))
A_K = 1.2                        # rms-norm scale folded into exp() scale arg
S_Q = float(1.0 / (HD * A_Q * A_Q))
B_Q = float(EPS / (A_Q * A_Q))
S_K = float(1.0 / (HD * A_K * A_K))
B_K = float(EPS / (A_K * A_K))
LN3I = float(np.log(1.0 / 3.0))

_compiled = {}


def _build(W):
    import concourse.bass as bass
    import concourse.tile as tile
    from concourse import bacc, bass_isa, mybir
    from concourse.masks import make_identity
    from contextlib import ExitStack

    f32 = mybir.dt.float32
    f32r = mybir.dt.float32r
    bf16 = mybir.dt.bfloat16
    AF = mybir.ActivationFunctionType
    OP = mybir.AluOpType

    NW = W // 128            # window in 128-tiles (8)
    assert W % 128 == 0

    nc = bacc.Bacc(None, target_bir_lowering=False)

    xT = nc.dram_tensor("xT", [C, T], bf16, kind="ExternalInput")
    wq = nc.dram_tensor("wqT", [C, HPC * HD], bf16, kind="ExternalInput")
    wk = nc.dram_tensor("wkT", [C, HD], bf16, kind="ExternalInput")
    wv = nc.dram_tensor("wvT", [C, HD], bf16, kind="ExternalInput")
    wp = nc.dram_tensor("wpT", [HPC * HD, C], bf16, kind="ExternalInput")
    wgd = nc.dram_tensor("wg", [GATE_CH, 1], bf16, kind="ExternalInput")
    ccd = nc.dram_tensor("cc", [HD, T], f32, kind="ExternalInput")
    ssd = nc.dram_tensor("ss", [HD, T], f32, kind="ExternalInput")
    ved = nc.dram_tensor("ve", [T, HD], f32, kind="ExternalInput")
    btrid = nc.dram_tensor("btri4", [128, HPC * 128], bf16, kind="ExternalInput")
    etrid = nc.dram_tensor("etri4", [128, HPC * 128], bf16, kind="ExternalInput")
    permd = nc.dram_tensor("perm", [128, 128], f32r, kind="ExternalInput")
    seld = nc.dram_tensor("sel", [128, 3], bf16, kind="ExternalInput")
    outT = nc.dram_tensor("outT", [C, T], f32, kind="ExternalOutput")

    with tile.TileContext(nc) as tc, ExitStack() as ctx:
        res = ctx.enter_context(tc.tile_pool(name="res", bufs=1))
        xs_p = ctx.enter_context(tc.tile_pool(name="xs", bufs=2))
        tab_p = ctx.enter_context(tc.tile_pool(name="tab", bufs=2))
        work_p = ctx.enter_context(tc.tile_pool(name="work", bufs=2))
        sq_p = ctx.enter_context(tc.tile_pool(name="sq", bufs=2))
        bc_p = ctx.enter_context(tc.tile_pool(name="bc", bufs=2))
        es_p = ctx.enter_context(tc.tile_pool(name="es", bufs=3))
        es3_p = ctx.enter_context(tc.tile_pool(name="es3", bufs=2))
        q4_p = ctx.enter_context(tc.tile_pool(name="q4", bufs=2))
        yt_p = ctx.enter_context(tc.tile_pool(name="yt", bufs=2))
        ot_p = ctx.enter_context(tc.tile_pool(name="ot", bufs=2))
        row_p = ctx.enter_context(tc.tile_pool(name="rows", bufs=2))

        ps_qkv = ctx.enter_context(tc.tile_pool(name="ps_qkv", bufs=2, space="PSUM"))
        ps_s = ctx.enter_context(tc.tile_pool(name="ps_s", bufs=2, space="PSUM"))
        ps_y = ctx.enter_context(tc.tile_pool(name="ps_y", bufs=1, space="PSUM"))
        ps_p = ctx.enter_context(tc.tile_pool(name="ps_p", bufs=2, space="PSUM"))
        ps_row = ctx.enter_context(tc.tile_pool(name="ps_row", bufs=1, space="PSUM"))
        dram_p = ctx.enter_context(tc.tile_pool(name="dram", bufs=2, space="DRAM"))

        # ---- resident tensors ----
        wg_sb = res.tile([GATE_CH, 1], bf16)
        nc.sync.dma_start(out=wg_sb, in_=wgd[:, :])
        wk_sb = res.tile([128, NCK, HD], bf16)
        xs0 = xs_p.tile([128, NCK, TS], bf16, tag="xs")
        wq_sb = res.tile([128, NCK, HPC * HD], bf16)
        # first parts small so the first k-proj matmuls can start early;
        # wq parts interleave so the q projections can start right after k
        def ldx(c0, c1):
            nc.sync.dma_start(
                out=xs0[:, c0:c1, :],
                in_=xT[128 * c0:128 * c1, 0:TS].rearrange(
                    "(c p) t -> p c t", p=128),
            )
        def ldwq(c0, c1):
            nc.sync.dma_start(
                out=wq_sb[:, c0:c1, :],
                in_=wq[128 * c0:128 * c1, :].rearrange(
                    "(c p) h -> p c h", p=128),
            )
        def ldwk(c0, c1):
            nc.sync.dma_start(
                out=wk_sb[:, c0:c1, :],
                in_=wk[128 * c0:128 * c1, :].rearrange(
                    "(c p) h -> p c h", p=128))
        ldwk(0, 1)
        ldx(0, 1)
        ldwk(1, 6)
        ldx(1, 6)
        ldwk(6, 16)
        ldx(6, 11)
        ldwq(0, 3)
        ldx(11, 16)
        ldwq(3, 8)
        ldwq(8, 12)
        ldwq(12, 16)
        wv_sb = res.tile([128, NCK, HD], bf16)
        nc.sync.dma_start(
            out=wv_sb, in_=wv[:, :].rearrange("(c p) h -> p c h", p=128))
        cc0 = tab_p.tile([128, TS], f32, tag="cc")
        nc.sync.dma_start(out=cc0, in_=ccd[:, 0:TS])
        ss0 = tab_p.tile([128, TS], f32, tag="ss")
        nc.sync.dma_start(out=ss0, in_=ssd[:, 0:TS])
        ve0 = tab_p.tile([128, TPS, HD], f32, tag="ve")
        nc.sync.dma_start(
            out=ve0, in_=ved[0:TS, :].rearrange("(tt p) h -> p tt h", p=128))
        btri_sb = res.tile([128, HPC * 128], bf16)
        nc.sync.dma_start(out=btri_sb, in_=btrid[:, :])
        etri_sb = res.tile([128, HPC * 128], bf16)
        nc.sync.dma_start(out=etri_sb, in_=etrid[:, :])
        perm_sb = res.tile([128, 128], f32r)
        nc.sync.dma_start(out=perm_sb, in_=permd[:, :])
        sel_sb = res.tile([128, 3], bf16)
        nc.sync.dma_start(out=sel_sb, in_=seld[:, :])
        wp_sb = res.tile([128, HPC, C], bf16)
        for h in range(HPC):
            nc.sync.dma_start(out=wp_sb[:, h, :], in_=wp[h * 128:(h + 1) * 128, :])

        ident = res.tile([128, 128], f32)
        make_identity(nc, ident)
        ones_sb = res.tile([128, 1], bf16)
        nc.vector.memset(ones_sb, 1.0)
        bq_sb = res.tile([128, 1], f32)
        nc.vector.memset(bq_sb, B_Q)
        bk_sb = res.tile([128, 1], f32)
        nc.vector.memset(bk_sb, B_K)
        bg_sb = res.tile([1, 1], f32)
        nc.vector.memset(bg_sb, LN3I)

        rows_ab = []
        for i in range(2):
            r = res.tile([128, 3, TS], bf16, tag=f"rows{i}")
            nc.vector.memset(r, 0.0)
            rows_ab.append(r)
        kT_sb = res.tile([128, T], f32r)        # rotated+normalized k, HD on partitions
        vn_sb = res.tile([128, NTT, HD], bf16)  # v natural, token tiles on partitions

        tabs = {0: (cc0, ss0, ve0)}
        xss = {0: xs0}
        yt_tiles = {}
        q4s = {}
        gates = {}

        def rope_half(dst_f32r, cc_sl, ss_sl, tag, scale_bc=None):
            """dst [128, TS] f32r pre-rotation. In-place RoPE; the half-swap
            runs as a PE permutation matmul (no DMA latency). The final write
            goes through the f32r view (required by consuming f32r matmuls)."""
            dst = dst_f32r.bitcast(f32)
            psw = ps_s.tile([128, HPC * 128], f32, tag="s")
            nc.tensor.matmul(psw[:, 0:TS], perm_sb, dst_f32r,
                             start=True, stop=True)
            tmp = work_p.tile([128, TS], f32, tag=tag + "t")
            nc.vector.tensor_mul(tmp, psw[:, 0:TS], ss_sl)
            nc.vector.tensor_mul(dst_f32r, dst, cc_sl)
            nc.vector.tensor_add(dst_f32r, dst, tmp)
            if scale_bc is not None:
                nc.vector.tensor_mul(dst_f32r, dst, scale_bc)

        def emit_cproj(m, j):
            """c_proj for q-subtile j of slice m (yt_tiles[(m, j)] ready)."""
            yt4 = yt_tiles.pop((m, j))
            t0 = m * TS
            ot = ot_p.tile([128, NTT, 128], f32, tag="ot")
            for gco in range(4):
                pp = ps_p.tile([128, 4 * 128], f32, tag="pp")
                for ci in range(4):
                    co = 4 * gco + ci
                    for h in range(HPC):
                        nc.tensor.matmul(
                            pp[:, ci * 128:(ci + 1) * 128],
                            wp_sb[:, h, co * 128:(co + 1) * 128],
                            yt4[:, h, :],
                            start=(h == 0), stop=(h == HPC - 1))
                if gco % 2 == 0:
                    nc.scalar.activation(ot[:, 4 * gco:4 * gco + 4, :], pp, AF.Copy)
                else:
                    nc.vector.tensor_copy(ot[:, 4 * gco:4 * gco + 4, :], pp)
                nc.sync.dma_start(
                    out=outT[512 * gco:512 * (gco + 1),
                             t0 + j * 128:t0 + (j + 1) * 128].rearrange(
                        "(co p) t -> p co t", p=128),
                    in_=ot[:, 4 * gco:4 * gco + 4, :])

        def emit_k(mm):
            """k projection + rms + rope for slice mm."""
            t0 = mm * TS
            xs = xss[mm]
            cc_sl, ss_sl, _ = tabs[mm]
            ps_k = ps_qkv.tile([128, TS], f32, tag="qkv")
            for c in range(NCK):
                nc.tensor.matmul(ps_k, wk_sb[:, c, :], xs[:, c, :],
                                 start=(c == 0), stop=(c == NCK - 1))
            sq_k = sq_p.tile([128, TS], f32, tag="sq")
            nc.scalar.activation(sq_k, ps_k, AF.Square)
            # rsqrt(mean(k^2)) broadcast across partitions; k is pre-normalized
            # (scale folded into the rope's final multiply), so the exp() scale
            # becomes the constant 1.0.
            rbk = bc_p.tile([128, TS], f32, tag="bc")
            nc.gpsimd.partition_all_reduce(rbk, sq_k, channels=128,
                                           reduce_op=bass_isa.ReduceOp.add)
            nc.scalar.activation(rbk, rbk, AF.Ln, bias=bk_sb, scale=S_K)
            nc.scalar.activation(rbk, rbk, AF.Exp, scale=-0.5)
            k_sl = kT_sb[:, t0:t0 + TS]
            nc.vector.tensor_copy(k_sl, ps_k)
            rope_half(k_sl, cc_sl, ss_sl, "ksw", scale_bc=rbk)

        def emit_gate(mm):
            """gate row: 3*sigmoid(x[:, :12] @ wg)."""
            xs = xss[mm]
            ps_g = ps_row.tile([1, TS], f32, tag="rows")
            nc.tensor.matmul(ps_g, wg_sb, xs[0:GATE_CH, 0, :], start=True,
                             stop=True)
            g_row = row_p.tile([1, TS], f32, tag="grow")
            # e^(-x)/3, then +1/3, then reciprocal => 3*sigmoid(x)
            nc.scalar.activation(g_row, ps_g, AF.Exp, scale=-1.0, bias=bg_sb)
            nc.vector.tensor_scalar(out=g_row, in0=g_row, scalar1=1.0 / 3.0,
                                    scalar2=None, op0=OP.add)
            nc.vector.reciprocal(g_row, g_row)
            g_dr = dram_p.tile([TS], f32, tag="gdr")
            nc.sync.dma_start(out=g_dr, in_=g_row)
            gate_c = row_p.tile([128, TPS], f32, tag="gate")
            nc.sync.dma_start(
                out=gate_c,
                in_=bass.AP(tensor=g_dr.tensor, offset=g_dr.offset,
                            ap=[[1, 128], [128, TPS]]),
            )
            gates[mm] = gate_c

        def emit_q(mm, h):
            """one q head: projection + rms-norm + rope. The rms-norm scale is
            applied as the LAST rope step so the rotation can proceed in
            parallel with the row chain."""
            xs = xss[mm]
            cc_sl, ss_sl, _ = tabs[mm]
            if h == 0:
                q4new = q4_p.tile([128, HPC, TS], f32r, tag="q4")
                q4s[mm] = q4new
            q4 = q4s[mm]
            ps_q = ps_qkv.tile([128, TS], f32, tag="qkv")
            for c in range(NCK):
                nc.tensor.matmul(ps_q, wq_sb[:, c, h * HD:(h + 1) * HD],
                                 xs[:, c, :],
                                 start=(c == 0), stop=(c == NCK - 1))
            nc.vector.tensor_copy(q4[:, h, :], ps_q)
            sq_q = sq_p.tile([128, TS], f32, tag="sq")
            nc.scalar.activation(sq_q, ps_q, AF.Square)
            rbc = bc_p.tile([128, TS], f32, tag="bc")
            nc.gpsimd.partition_all_reduce(rbc, sq_q,
                                           channels=128,
                                           reduce_op=bass_isa.ReduceOp.add)
            nc.scalar.activation(rbc, rbc, AF.Ln, bias=bq_sb, scale=S_Q)
            nc.scalar.activation(rbc, rbc, AF.Exp, scale=-0.5)
            rope_half(q4[:, h, :], cc_sl, ss_sl, "qsw", scale_bc=rbc)

        def emit_v(mm):
            """v projection + transpose to natural + gate-add; last user of
            xs/tabs/gate for slice mm."""
            xs = xss.pop(mm)
            _, _, ve_sl = tabs.pop(mm)
            gate_c = gates.pop(mm)
            ps_v = ps_qkv.tile([128, TS], f32, tag="qkv")
            for c in range(NCK):
                nc.tensor.matmul(ps_v, wv_sb[:, c, :], xs[:, c, :],
                                 start=(c == 0), stop=(c == NCK - 1))
            vT_s = work_p.tile([128, TS], f32, tag="vt")
            nc.scalar.activation(vT_s, ps_v, AF.Copy)
            ps_t = ps_qkv.tile([128, TS], f32, tag="qkv")
            for tt in range(TPS):
                nc.tensor.transpose(ps_t[:, tt * 128:(tt + 1) * 128],
                                    vT_s[:, tt * 128:(tt + 1) * 128], ident)
            # evacuate the transposes immediately so the PSUM bank frees
            # without waiting for the gate_c DRAM roundtrip
            vN_s = work_p.tile([128, TS], f32, tag="vn")
            nc.scalar.activation(vN_s, ps_t, AF.Copy)
            for tt in range(TPS):
                nc.vector.scalar_tensor_tensor(
                    out=vn_sb[:, mm * TPS + tt, :],
                    in0=ve_sl[:, tt, :], scalar=gate_c[:, tt:tt + 1],
                    in1=vN_s[:, tt * 128:(tt + 1) * 128],
                    op0=OP.mult, op1=OP.add)

        def emit_prefetch(mm):
            """issue the xs/cc/ss/ve loads for slice mm."""
            t1 = mm * TS
            xs_n = xs_p.tile([128, NCK, TS], bf16, tag="xs")
            for p4 in range(4):
                nc.sync.dma_start(
                    out=xs_n[:, 4 * p4:4 * p4 + 4, :],
                    in_=xT[512 * p4:512 * (p4 + 1), t1:t1 + TS].rearrange(
                        "(c p) t -> p c t", p=128),
                )
            cc_n = tab_p.tile([128, TS], f32, tag="cc")
            nc.sync.dma_start(out=cc_n, in_=ccd[:, t1:t1 + TS])
            ss_n = tab_p.tile([128, TS], f32, tag="ss")
            nc.sync.dma_start(out=ss_n, in_=ssd[:, t1:t1 + TS])
            ve_n = tab_p.tile([128, TPS, HD], f32, tag="ve")
            nc.sync.dma_start(
                out=ve_n,
                in_=ved[t1:t1 + TS, :].rearrange("(tt p) h -> p tt h", p=128))
            xss[mm] = xs_n
            tabs[mm] = (cc_n, ss_n, ve_n)

        # ---- slice 0 QKV up front (nothing to interleave into yet) ----
        emit_k(0)
        emit_gate(0)
        for h in range(HPC):
            emit_q(0, h)
        emit_prefetch(1)
        emit_v(0)

        for m in range(NSL):
            t0 = m * TS
            q4 = q4s[m]

            # ---- attention subtiles, interleaved with c_proj of the previous
            # subtile and the NEXT slice's QKV phases (PE filler during the
            # ACT-paced attention stretch) ----
            for j in range(TPS):
                t = m * TPS + j
                nlo = max(0, t - NW)
                # Order: one full tile opens the PSUM groups (short dep chain),
                # the masked boundary tiles (diag/edge) go next so their longer
                # exp->mask chains overlap the remaining full tiles' work.
                fulls = [n for n in range(nlo, t + 1)
                         if n != t and n != t - NW]
                ns = []
                if fulls:
                    ns.append(fulls[0])
                ns.append(t)                      # diag (btri)
                if t - NW >= 0:
                    ns.append(t - NW)             # edge (etri)
                ns.extend(fulls[1:])
                L = len(ns)
                last = L - 1
                # hybrid denominator: the first `early` tiles are summed on
                # Pool (their reduces finish well before the subtile ends);
                # the last 3 tiles keep PE ones-matmuls so the chain stays
                # short. sel-matmuls fold the Pool partials into ps_sum.
                early = L - 3 if L >= 6 else 0
                G = (early + 2) // 3
                gsz = [min(3, early - 3 * g) for g in range(G)]
                rows = rows_ab[(m * TPS + j) % 2]
                psy = ps_y.tile([128, HPC * 128], f32, tag="py")
                ps_sum = ps_row.tile([1, TS], f32, tag="rows")
                esg = []
                for _g in range(G):
                    es3 = es3_p.tile([128, 3, HPC * 128], bf16, tag="es3")
                    esg.append(es3)
                for idx, n in enumerate(ns):
                    pss = ps_s.tile([128, HPC * 128], f32, tag="s")
                    nc.tensor.matmul(pss, kT_sb[:, n * 128:(n + 1) * 128],
                                     q4[:, :, j * 128:(j + 1) * 128],
                                     start=True, stop=True)
                    if idx < early:
                        g, gi = idx // 3, idx % 3
                        es = esg[g][:, gi, :]
                    else:
                        es = es_p.tile([128, HPC * 128], bf16, tag="es")
                    nc.scalar.activation(es, pss, AF.Exp)
                    if n == t:
                        nc.vector.tensor_mul(es, es, btri_sb)
                    if n == t - NW:
                        nc.vector.tensor_mul(es, es, etri_sb)
                    if idx >= early:
                        nc.tensor.matmul(ps_sum, ones_sb, es,
                                         start=(idx == early),
                                         stop=(G == 0 and idx == last))
                    nc.tensor.matmul(psy, vn_sb[:, n, :], es,
                                     start=(idx == 0), stop=(idx == last))
                    if idx < early and (idx % 3 == gsz[idx // 3] - 1):
                        g = idx // 3
                        with nc.allow_low_precision(
                                reason="f32r rows: softmax denominator "
                                       "tolerates TF32-width rounding"):
                            nc.gpsimd.tensor_reduce(
                                rows[32 * g:32 * g + 1, 0:gsz[g], :],
                                esg[g][:, 0:gsz[g], :],
                                axis=mybir.AxisListType.C, op=OP.add)
                if G:
                    for b in range(gsz[0]):
                        cnt = sum(1 for x in gsz if x > b)
                        nc.tensor.matmul(ps_sum, sel_sb[:, cnt - 1:cnt],
                                         rows[:, b, :], start=False,
                                         stop=(b == gsz[0] - 1))
                rsum = row_p.tile([1, TS], f32, tag="rsum")
                nc.vector.reciprocal(rsum, ps_sum)
                sbc = bc_p.tile([128, TS], f32, tag="sbc")
                nc.gpsimd.partition_broadcast(sbc, rsum)
                yt4 = yt_p.tile([128, HPC, 128], bf16, tag="yt")
                nc.vector.tensor_mul(
                    yt4.rearrange("p h t -> p (h t)"), psy, sbc)
                yt_tiles[(m, j)] = yt4
                if j > 0:
                    emit_cproj(m, j - 1)
                else:
                    if m > 0:
                        emit_cproj(m - 1, TPS - 1)
                if m + 1 < NSL:
                    if j == 0:
                        emit_k(m + 1)
                        emit_gate(m + 1)
                    elif j == 1:
                        emit_q(m + 1, 0)
                        emit_q(m + 1, 1)
                        if m + 2 < NSL:
                            emit_prefetch(m + 2)
                    elif j == 2:
                        emit_q(m + 1, 2)
                        emit_q(m + 1, 3)
                    else:
                        emit_v(m + 1)
            del q4s[m]

        emit_cproj(NSL - 1, TPS - 1)

    # Restrict the activation-table picker to the one set containing every
    # ACT function we use (exp, ln, square, copy): without this the greedy
    # picker alternates tables, inserting a ~1.3us table load per switch.
    import concourse.hw_specs as hw_specs
    import concourse.bacc as bacc_mod

    orig = hw_specs.get_activation_tables

    def only_combined(arch):
        t = orig(arch)
        return {k: (v if k == "natural_log_exp_and_others" else set())
                for k, v in t.items()}

    hw_specs.get_activation_tables = only_combined
    bacc_mod.get_activation_tables = only_combined
    try:
        nc.compile()
    finally:
        hw_specs.get_activation_tables = orig
        bacc_mod.get_activation_tables = orig
    return nc


def _prep_inputs(x, ve, cos, sin, Wq, Wk, Wv, Wproj, Wgate, W):
    import ml_dtypes

    bf = ml_dtypes.bfloat16
    cosT = np.ascontiguousarray(cos[0, :, 0, :].T)  # (64, T)
    sinT = np.ascontiguousarray(sin[0, :, 0, :].T)
    cc = np.concatenate([cosT, cosT], axis=0).astype(np.float32)
    ss = np.concatenate([sinT, -sinT], axis=0).astype(np.float32)
    p = np.arange(128)[:, None]
    f = np.arange(128)[None, :]
    btri = (p <= f).astype(np.float32)
    etri = (f <= p + (W % 128)).astype(np.float32)
    btri4 = np.ascontiguousarray(np.tile(btri, (1, HPC)))
    etri4 = np.ascontiguousarray(np.tile(etri, (1, HPC)))
    # half-swap permutation: out[p] = in[(p+64) % 128]
    perm = np.zeros((128, 128), dtype=np.float32)
    perm[(np.arange(128) + 64) % 128, np.arange(128)] = 1.0
    sel = np.zeros((128, 3), dtype=np.float32)
    for c in range(3):
        sel[[32 * g for g in range(c + 1)], c] = 1.0

    in_maps = []
    for core in range(8):
        b, g = core // NKV, core % NKV
        hs = slice(g * HPC * HD, (g + 1) * HPC * HD)
        ks = slice(g * HD, (g + 1) * HD)
        in_maps.append({
            "xT": np.ascontiguousarray(x[b].T).astype(bf),
            "wqT": np.ascontiguousarray(Wq[hs, :].T).astype(bf),
            "wkT": np.ascontiguousarray(Wk[ks, :].T).astype(bf),
            "wvT": np.ascontiguousarray(Wv[ks, :].T).astype(bf),
            "wpT": np.ascontiguousarray(Wproj[:, hs].T).astype(bf),
            "wg": np.ascontiguousarray(Wgate[g][:, None]).astype(bf),
            "cc": cc,
            "ss": ss,
            "ve": np.ascontiguousarray(ve[b][:, ks]),
            "btri4": btri4.astype(bf),
            "etri4": etri4.astype(bf),
            "perm": perm,
            "sel": sel.astype(bf),
        })
    return in_maps


def _run(inputs, trace=False):
    from concourse.bass_utils import run_bass_kernel_spmd

    x = np.asarray(inputs["x"], dtype=np.float32)
    ve = np.asarray(inputs["ve"], dtype=np.float32)
    cos = np.asarray(inputs["cos"], dtype=np.float32)
    sin = np.asarray(inputs["sin"], dtype=np.float32)
    Wq = np.asarray(inputs["Wq"], dtype=np.float32)
    Wk = np.asarray(inputs["Wk"], dtype=np.float32)
    Wv = np.asarray(inputs["Wv"], dtype=np.float32)
    Wproj = np.asarray(inputs["Wproj"], dtype=np.float32)
    Wgate = np.asarray(inputs["Wgate"], dtype=np.float32)
    W = int(inputs["window_size"])

    if W not in _compiled:
        _compiled[W] = _build(W)
    nc = _compiled[W]

    in_maps = _prep_inputs(x, ve, cos, sin, Wq, Wk, Wv, Wproj, Wgate, W)
    res = run_bass_kernel_spmd(nc, in_maps, core_ids=list(range(8)), trace=trace)

    out = np.zeros((B, T, C), dtype=np.float32)
    for core in range(8):
        b = core // NKV
        out[b] += res.results[core]["outT"].T
    return out, res


def kernel(**inputs):
    out, _ = _run(inputs, trace=False)
    return out
